# revision 6
# baseline (speedup 1.0000x reference)
"""MultiHeadAttention (pre-LN, residual) Trainium2 Bass kernel, 8 NeuronCores.

Problem: q,k,v [2, 2048, 1024], 16 heads x 64 dim, LN(q) -> QKV proj ->
softmax attention -> out proj -> +residual(q).

Sharding (v3): core c owns 256 tokens of EACH batch (512 total).  All
projections are token-sharded.  K / V projections are AllGathered across
ALL 8 cores (8-rank gather => mesh algorithm + Shared output), so every
core holds both batches' full K^T / V and computes attention + output
projection for its own 512 query tokens.

Schedule: K proj runs dt-outer (8 psum accumulators) so it starts as soon
as the first weight slice lands; its result is stored + gathered in two
512-feature chunks.  LN/Q-proj overlap the K gather.  The attention loop
interleaves, per key-tile group g: S matmuls for head-pair hp+2 and O
matmuls for hp, so ScalarE (exp, the serial bottleneck: ~147us total)
never starves while the PE does O work.  The V projection is interleaved
into the first prologue (PE is otherwise idle while exp(hp0) runs), and
its gather overlaps the first two exp windows.

PE layout: "T layout" = features on partitions, tokens on free axis.
  S^T tile [keys, q] = matmul(lhsT=K_h^T [64, keys], rhs=Q_h^T [64, q])
    -- the two heads of a pair run CONCURRENTLY as row-tiled
       64-contraction matmuls at PE tile positions (0,0) / (64,0),
       writing different PSUM banks.
  O^T [dv+1, q]     += matmul(lhsT=[V|1] [keys, 65], rhs=exp(S^T))
Softmax is unnormalized exp (S/tau ~ N(0,1)); psum row 64 accumulates the
denominator, reciprocated and broadcast via a K=1 ones matmul.
"""

import numpy as np

N_CORES = 8
B, L, D = 2, 2048, 1024
H, DK, DV = 16, 64, 64
TPB = L // N_CORES    # 256 tokens per core per batch
TPC = B * TPB         # 512 tokens per core total
P = 128
NDT = D // P          # 8 d-tiles of 128
NMT = D // P          # 8 output-feature tiles
NTT = TPC // P        # 4 token tiles of 128 per core
NKT = L // P          # 16 key tiles of 128 per batch
NHP = H // 2          # 8 head pairs
NG = 16               # attention inner groups per head pair: (b, ktp)
EPS = 1e-6
TAU_INV = 1.0 / float(np.sqrt(DK))

_CACHE = {}


def _np_reference(q, k, v, mask, w_q, w_k, w_v, w_o, ln_g, ln_b):
    """Pure-numpy fallback (only used if mask isn't all-ones)."""
    q64 = q.astype(np.float64)
    mu = q64.mean(-1, keepdims=True)
    var = q64.var(-1, keepdims=True)
    qn = (q64 - mu) / np.sqrt(var + EPS) * ln_g + ln_b
    Q = (qn @ w_q.T.astype(np.float64)).reshape(B, L, H, DK).transpose(0, 2, 1, 3)
    K = (k.astype(np.float64) @ w_k.T.astype(np.float64)).reshape(B, L, H, DK).transpose(0, 2, 1, 3)
    V = (v.astype(np.float64) @ w_v.T.astype(np.float64)).reshape(B, L, H, DV).transpose(0, 2, 1, 3)
    S = np.einsum("bhqd,bhkd->bhqk", Q / np.sqrt(DK), K)
    S = np.where(mask[None, None] == 0, -1e9, S)
    S = S - S.max(-1, keepdims=True)
    Pm = np.exp(S)
    Pm = Pm / Pm.sum(-1, keepdims=True)
    O = np.einsum("bhqk,bhkd->bhqd", Pm, V)
    O = O.transpose(0, 2, 1, 3).reshape(B, L, H * DV)
    out = O @ w_o.T.astype(np.float64) + q64
    return out.astype(np.float32)


def build_nc():
    import concourse.bass as bass
    import concourse.mybir as mybir
    import concourse.tile as tile
    from concourse import bacc
    from concourse.masks import make_identity

    f32 = mybir.dt.float32
    bf16 = mybir.dt.bfloat16

    nc = bacc.Bacc(num_devices=N_CORES)

    q_c = nc.declare_dram_parameter("q_c", [TPC, D], bf16, isOutput=False)
    kT_c = nc.declare_dram_parameter("kT_c", [D, TPC], bf16, isOutput=False)
    vT_c = nc.declare_dram_parameter("vT_c", [D, TPC], bf16, isOutput=False)
    wgqT = nc.declare_dram_parameter("wgqT", [D, D], bf16, isOutput=False)
    wkT = nc.declare_dram_parameter("wkT", [D, D], bf16, isOutput=False)
    wvT = nc.declare_dram_parameter("wvT", [D, D], bf16, isOutput=False)
    woT = nc.declare_dram_parameter("woT", [D, D], bf16, isOutput=False)
    cq = nc.declare_dram_parameter("cq", [D], f32, isOutput=False)
    out_c = nc.declare_dram_parameter("out_c", [TPC, D], f32, isOutput=True)

    RG = [list(range(N_CORES))]

    with tile.TileContext(nc) as tc:
        with tc.tile_pool(name="dram", bufs=1, space="DRAM") as dram:
            kag_in = [dram.tile([D // 2, TPC], bf16, name=f"kag_in{c}")
                      for c in range(2)]
            vag_in = dram.tile([TPC, D], bf16)
            kag_out = [
                dram.tile([N_CORES, D // 2, TPC], bf16, name=f"kag_out{c}",
                          addr_space="Shared")
                for c in range(2)
            ]
            vag_out = dram.tile([N_CORES, TPC, D], bf16, addr_space="Shared")

            with tc.tile_pool(name="singles", bufs=1) as singles:
                ident = singles.tile([P, P], f32)
                make_identity(nc, ident)
                ones_f32 = singles.tile([P, DK], f32)
                nc.vector.memset(ones_f32, 1.0)
                eps_sb = singles.tile([P, 1], f32)
                nc.vector.memset(eps_sb, EPS)
                cq_sb = singles.tile([P, NMT], f32)
                nc.sync.dma_start(out=cq_sb, in_=cq.rearrange("(mt p) -> p mt", p=P))

                with tc.tile_pool(name="persist", bufs=1) as persist:
                    q_sb = persist.tile([P, NTT, D], bf16)       # residual + LN in
                    qT_sb = persist.tile([P, NMT, TPC], bf16)    # Q^T
                    aO_sb = persist.tile([P, NHP, TPC], bf16)    # attn out^T

                    # ============ Phase 1: K projection (dt-outer) + gather ===
                    with tc.tile_pool(name="p1", bufs=1) as p1, \
                         tc.tile_pool(name="p1psum", bufs=1, space="PSUM") as p1psum:
                        wk_sb = p1.tile([P, NDT, D], bf16)
                        ktc_sb = p1.tile([P, NDT, TPC], bf16)
                        wkr = wkT.rearrange("(dt p) m -> p dt m", p=P)
                        ktr = kT_c.rearrange("(dt p) t -> p dt t", p=P)
                        for dt in range(NDT):
                            nc.sync.dma_start(out=wk_sb[:, dt, :], in_=wkr[:, dt, :])
                            nc.sync.dma_start(out=ktc_sb[:, dt, :], in_=ktr[:, dt, :])
                        nc.sync.dma_start(
                            out=q_sb, in_=q_c.rearrange("(tt p) d -> p tt d", p=P)
                        )
                        kps = [
                            p1psum.tile([P, TPC], f32, tag=f"kp{mt}",
                                        name=f"kps{mt}")
                            for mt in range(NMT)
                        ]
                        for dt in range(NDT):
                            for mt in range(NMT):
                                nc.tensor.matmul(
                                    kps[mt],
                                    wk_sb[:, dt, mt * P:(mt + 1) * P],
                                    ktc_sb[:, dt, :],
                                    start=(dt == 0),
                                    stop=(dt == NDT - 1),
                                )
                        kc_sb = p1.tile([P, NMT, TPC], bf16)
                        for mt in range(NMT):
                            nc.vector.tensor_copy(kc_sb[:, mt, :], kps[mt])
                            c = mt // 4
                            r = mt % 4
                            nc.gpsimd.dma_start(
                                out=kag_in[c][r * P:(r + 1) * P, :],
                                in_=kc_sb[:, mt, :],
                            )
                            if r == 3:
                                nc.gpsimd.collective_compute(
                                    "AllGather",
                                    mybir.AluOpType.bypass,
                                    replica_groups=RG,
                                    ins=[kag_in[c][:, :].opt()],
                                    outs=[kag_out[c][:, :, :].opt()],
                                )

                    # ============ Phase 2: LayerNorm + Q projection ===========
                    with tc.tile_pool(name="p2", bufs=1) as p2, \
                         tc.tile_pool(name="p2s", bufs=4) as p2s, \
                         tc.tile_pool(name="p2psum", bufs=3, space="PSUM") as p2psum, \
                         tc.tile_pool(name="tpsum", bufs=2, space="PSUM") as tpsum:
                        wq_sb = p2.tile([P, NDT, D], bf16)
                        nc.sync.dma_start(
                            out=wq_sb, in_=wgqT.rearrange("(dt p) m -> p dt m", p=P)
                        )
                        qn_sb = p2.tile([P, NTT, D], f32)
                        for tt in range(NTT):
                            stats = p2s.tile([P, 2, 6], f32)
                            for sg in range(2):
                                nc.vector.bn_stats(
                                    out=stats[:, sg, :],
                                    in_=q_sb[:, tt, sg * 512:(sg + 1) * 512],
                                )
                            mv = p2s.tile([P, 2], f32)
                            nc.vector.bn_aggr(out=mv, in_=stats)
                            rstd = p2s.tile([P, 1], f32)
                            nc.scalar.activation(
                                out=rstd,
                                in_=mv[:, 1:2],
                                func=mybir.ActivationFunctionType.Sqrt,
                                bias=eps_sb,
                                scale=1.0,
                            )
                            nc.vector.reciprocal(out=rstd, in_=rstd)
                            nc.vector.tensor_scalar(
                                out=qn_sb[:, tt, :],
                                in0=q_sb[:, tt, :],
                                scalar1=mv[:, 0:1],
                                scalar2=rstd,
                                op0=mybir.AluOpType.subtract,
                                op1=mybir.AluOpType.mult,
                            )

                        qnT_sb = p2.tile([P, NDT, TPC], bf16)
                        for tt in range(NTT):
                            for dt in range(NDT):
                                tp = tpsum.tile([P, P], f32, tag="tp")
                                nc.tensor.transpose(
                                    tp, qn_sb[:, tt, dt * P:(dt + 1) * P], ident
                                )
                                nc.vector.tensor_copy(
                                    qnT_sb[:, dt, tt * P:(tt + 1) * P], tp
                                )

                        for mt in range(NMT):
                            ps = p2psum.tile([P, TPC], f32, tag="qps")
                            for dt in range(NDT):
                                nc.tensor.matmul(
                                    ps,
                                    wq_sb[:, dt, mt * P:(mt + 1) * P],
                                    qnT_sb[:, dt, :],
                                    start=(dt == 0),
                                    stop=(dt == NDT - 1),
                                )
                            nc.vector.tensor_scalar(
                                out=qT_sb[:, mt, :],
                                in0=ps,
                                scalar1=cq_sb[:, mt:mt + 1],
                                scalar2=None,
                                op0=mybir.AluOpType.add,
                            )

                    # ============ Phase 3: attention (V proj in prologue) =====
                    with tc.tile_pool(name="pv", bufs=1) as pv, \
                         tc.tile_pool(name="vnp", bufs=2) as vnp, \
                         tc.tile_pool(name="kv", bufs=1) as kvp, \
                         tc.tile_pool(name="es", bufs=1) as es, \
                         tc.tile_pool(name="rp", bufs=2) as rp, \
                         tc.tile_pool(name="spsum", bufs=2, space="PSUM") as spsum, \
                         tc.tile_pool(name="opsum", bufs=2, space="PSUM") as opsum:
                        wv_sb = pv.tile([P, NDT, D], bf16)
                        vtc_sb = pv.tile([P, NDT, TPC], bf16)
                        wvr = wvT.rearrange("(dt p) m -> p dt m", p=P)
                        vtr = vT_c.rearrange("(dt p) t -> p dt t", p=P)
                        for dt in range(NDT):
                            nc.sync.dma_start(out=wv_sb[:, dt, :], in_=wvr[:, dt, :])
                            nc.sync.dma_start(out=vtc_sb[:, dt, :], in_=vtr[:, dt, :])

                        ksb_bufs = []
                        vsb_bufs = []
                        est_bufs = []
                        for i in range(2):
                            # h on partition halves only: [64h:(h+1)64, b, kt, key]
                            kb = kvp.tile([P, B, NKT, P], bf16, name=f"ksb{i}")
                            # [key part, b, kt, h, 65 = V | ones]
                            vb = kvp.tile([P, B, NKT, 2, 65], bf16, name=f"vsb{i}")
                            for h in range(2):
                                nc.vector.memset(vb[:, :, :, h, DV:DV + 1], 1.0)
                            ksb_bufs.append(kb)
                            vsb_bufs.append(vb)
                        for i in range(3):
                            # [keys part, h, kt, b, 256]
                            eb = es.tile([P, 2, NKT, B, TPB], bf16, name=f"est{i}")
                            est_bufs.append(eb)

                        kag_r = [
                            kag_out[c].rearrange(
                                "r f (b t c2) -> f b t r c2", b=B, c2=P
                            )  # [512, B, 2, 8 ranks, 128]
                            for c in range(2)
                        ]

                        def emit_k_loads(hp):
                            ksb = ksb_bufs[hp % 2]
                            ch = hp // 4
                            f0 = (hp % 4) * P
                            for h in range(2):
                                for b in range(B):
                                    for t in range(2):
                                        nc.sync.dma_start(
                                            out=ksb[
                                                h * DK:(h + 1) * DK, b,
                                                t * 8:(t + 1) * 8, :,
                                            ],
                                            in_=kag_r[ch][
                                                f0 + h * DK:f0 + (h + 1) * DK,
                                                b, t, :, :,
                                            ],
                                        )

                        def emit_v_loads(hp):
                            vsb = vsb_bufs[hp % 2]
                            for h in range(2):
                                for b in range(B):
                                    for t in range(2):
                                        nc.sync.dma_start(
                                            out=vsb[:, b, t * 8:(t + 1) * 8, h, 0:DV],
                                            in_=vag_out[
                                                :,
                                                b * TPB + t * P:b * TPB + (t + 1) * P,
                                                hp * P + h * DK:hp * P + (h + 1) * DK,
                                            ].rearrange("r p c -> p r c"),
                                        )

                        def s_group(hp, g):
                            b, ktp = divmod(g, NG // 2)
                            ksb = ksb_bufs[hp % 2]
                            est = est_bufs[hp % 3]
                            ps = spsum.tile(
                                [P, 2, 2, TPB], f32, tag="s",
                                name=f"s_{hp}_{g}",
                            )
                            for half in range(2):
                                kt = 2 * ktp + half
                                for h in range(2):
                                    nc.tensor.matmul(
                                        ps[:, h, half, :],
                                        ksb[h * DK:(h + 1) * DK, b, kt, :],
                                        qT_sb[
                                            h * DK:(h + 1) * DK, hp,
                                            b * TPB:(b + 1) * TPB,
                                        ],
                                        start=True,
                                        stop=True,
                                    )
                            nc.scalar.activation(
                                out=est[:, :, 2 * ktp:2 * ktp + 2, b, :],
                                in_=ps,
                                func=mybir.ActivationFunctionType.Exp,
                                scale=TAU_INV,
                            )

                        def o_chunk(hp, g, oAB):
                            b, ktp = divmod(g, NG // 2)
                            vsb = vsb_bufs[hp % 2]
                            est = est_bufs[hp % 3]
                            for half in range(2):
                                kt = 2 * ktp + half
                                for h in range(2):
                                    nc.tensor.matmul(
                                        oAB[h][0:DV + 1, b, :],
                                        vsb[:, b, kt, h, :],
                                        est[:, h, kt, b, :],
                                        start=(ktp == 0 and half == 0),
                                        stop=(ktp == NG // 2 - 1 and half == 1),
                                    )

                        def v_chunk(c):
                            tt, mc = divmod(c, 2)
                            ps_v = spsum.tile(
                                [P, 2, 2, TPB], f32, tag="s", name=f"vps{c}"
                            )
                            pv_view = ps_v[:, 0, :, :]
                            for dt in range(NDT):
                                nc.tensor.matmul(
                                    pv_view,
                                    vtc_sb[:, dt, tt * P:(tt + 1) * P],
                                    wv_sb[:, dt, mc * 512:(mc + 1) * 512],
                                    start=(dt == 0),
                                    stop=(dt == NDT - 1),
                                )
                            vn = vnp.tile([P, 2, TPB], bf16, tag="vn",
                                          name=f"vn{c}")
                            nc.vector.tensor_copy(vn, pv_view)
                            nc.gpsimd.dma_start(
                                out=vag_in[
                                    tt * P:(tt + 1) * P, mc * 512:(mc + 1) * 512
                                ].rearrange("p (a c2) -> p a c2", a=2),
                                in_=vn,
                            )

                        def emit_norm(hp, oAB):
                            rsb = rp.tile([P, 2, B, TPB], f32, tag="r",
                                          name=f"rsb{hp}")
                            for h in range(2):
                                for b in range(B):
                                    nc.vector.reciprocal(
                                        out=rsb[0:1, h, b, :],
                                        in_=oAB[h][DV:DV + 1, b, :],
                                    )
                            ps_rbc = spsum.tile([P, 2, 2, TPB], f32, tag="s",
                                                name=f"rbc{hp}")
                            for h in range(2):
                                for b in range(B):
                                    nc.tensor.matmul(
                                        ps_rbc[DK * h:DK * (h + 1), 0, b, :],
                                        ones_f32[0:1, :],
                                        rsb[0:1, h, b, :],
                                        start=True,
                                        stop=True,
                                        tile_position=(0, DK * h),
                                    )
                            rbc_sb = rp.tile([P, B, TPB], f32, tag="rbs",
                                             name=f"rbs{hp}")
                            nc.vector.tensor_copy(rbc_sb, ps_rbc[:, 0, :, :])
                            for h in range(2):
                                for b in range(B):
                                    nc.vector.tensor_mul(
                                        aO_sb[
                                            DK * h:DK * (h + 1), hp,
                                            b * TPB:(b + 1) * TPB,
                                        ],
                                        oAB[h][0:DV, b, :],
                                        rbc_sb[DK * h:DK * (h + 1), b, :],
                                    )

                        # ---- prologue ----
                        emit_k_loads(0)
                        emit_k_loads(1)
                        # S(0) + exp(0), with the V projection interleaved into
                        # the PE idle time while ScalarE chews on exp(0).
                        for g in range(NG):
                            s_group(0, g)
                            if g % 2 == 0:
                                v_chunk(g // 2)
                        nc.gpsimd.collective_compute(
                            "AllGather",
                            mybir.AluOpType.bypass,
                            replica_groups=RG,
                            ins=[vag_in[:, :].opt()],
                            outs=[vag_out[:, :, :].opt()],
                        )
                        emit_v_loads(0)
                        for g in range(NG):
                            s_group(1, g)
                        emit_v_loads(1)

                        # ---- steady state ----
                        norm_queue = []
                        for hp in range(NHP):
                            if hp + 2 < NHP:
                                emit_k_loads(hp + 2)
                            oAB = [
                                opsum.tile([P, B, TPB], f32, tag="oA",
                                           name=f"oA_{hp}"),
                                opsum.tile([P, B, TPB], f32, tag="oB",
                                           name=f"oB_{hp}"),
                            ]
                            for g in range(NG):
                                if hp + 2 < NHP:
                                    s_group(hp + 2, g)
                                o_chunk(hp, g, oAB)
                                if g == 4 and norm_queue:
                                    emit_norm(*norm_queue.pop())
                            if hp + 2 < NHP:
                                emit_v_loads(hp + 2)
                            norm_queue.append((hp, oAB))
                        emit_norm(*norm_queue.pop())

                    # ============ Phase 4: out projection + residual ==========
                    with tc.tile_pool(name="p4", bufs=1) as p4, \
                         tc.tile_pool(name="p4o", bufs=2) as p4o, \
                         tc.tile_pool(name="p4psum", bufs=2, space="PSUM") as p4psum:
                        wo_sb = p4.tile([P, NDT, D], bf16)
                        nc.sync.dma_start(
                            out=wo_sb, in_=woT.rearrange("(dt p) m -> p dt m", p=P)
                        )
                        for tt in range(NTT):
                            ob = p4o.tile([P, D], f32, tag="ob")
                            for mc in range(2):
                                ps = p4psum.tile([P, TPC], f32, tag="ops")
                                for dt in range(NDT):
                                    nc.tensor.matmul(
                                        ps,
                                        aO_sb[:, dt, tt * P:(tt + 1) * P],
                                        wo_sb[:, dt, mc * 512:(mc + 1) * 512],
                                        start=(dt == 0),
                                        stop=(dt == NDT - 1),
                                    )
                                nc.vector.tensor_add(
                                    ob[:, mc * 512:(mc + 1) * 512],
                                    ps,
                                    q_sb[:, tt, mc * 512:(mc + 1) * 512],
                                )
                            nc.sync.dma_start(
                                out=out_c[tt * P:(tt + 1) * P, :], in_=ob
                            )

    nc.compile()
    return nc


def _get_nc():
    if "nc" not in _CACHE:
        _CACHE["nc"] = build_nc()
    return _CACHE["nc"]


def make_in_maps(q, k, v, w_q, w_k, w_v, w_o, ln_g, ln_b):
    import ml_dtypes

    bf = ml_dtypes.bfloat16
    q2 = np.ascontiguousarray(q.reshape(B * L, D).astype(bf))
    kT = np.ascontiguousarray(k.reshape(B * L, D).T.astype(bf))
    vT = np.ascontiguousarray(v.reshape(B * L, D).T.astype(bf))
    wgqT = np.ascontiguousarray((w_q * ln_g[None, :]).T.astype(bf))
    wkT = np.ascontiguousarray(w_k.T.astype(bf))
    wvT = np.ascontiguousarray(w_v.T.astype(bf))
    woT = np.ascontiguousarray(w_o.T.astype(bf))
    cq = np.ascontiguousarray(w_q @ ln_b, dtype=np.float32)
    in_maps = []
    for c in range(N_CORES):
        i0 = c * TPB
        rows = np.concatenate(
            [np.arange(i0, i0 + TPB), L + np.arange(i0, i0 + TPB)]
        )
        in_maps.append(
            {
                "q_c": np.ascontiguousarray(q2[rows]),
                "kT_c": np.ascontiguousarray(kT[:, rows]),
                "vT_c": np.ascontiguousarray(vT[:, rows]),
                "wgqT": wgqT,
                "wkT": wkT,
                "wvT": wvT,
                "woT": woT,
                "cq": cq,
            }
        )
    return in_maps


def run(inputs, trace=False, tmpdir=None):
    """Run the device kernel.  Returns (out [B, L, D], BassKernelResults)."""
    from concourse.bass_utils import run_bass_kernel_spmd

    nc = _get_nc()
    in_maps = make_in_maps(
        inputs["q"], inputs["k"], inputs["v"], inputs["w_q"], inputs["w_k"],
        inputs["w_v"], inputs["w_o"], inputs["ln_g"], inputs["ln_b"],
    )
    res = run_bass_kernel_spmd(
        nc, in_maps, list(range(N_CORES)), trace=trace, tmpdir=tmpdir
    )
    out = np.empty((B * L, D), dtype=np.float32)
    for c in range(N_CORES):
        i0 = c * TPB
        oc = res.results[c]["out_c"]
        out[i0:i0 + TPB] = oc[0:TPB]
        out[L + i0:L + i0 + TPB] = oc[TPB:TPC]
    return out.reshape(B, L, D), res


def kernel(q, k, v, mask, w_q, w_k, w_v, w_o, ln_g, ln_b):
    q = np.asarray(q, dtype=np.float32)
    k = np.asarray(k, dtype=np.float32)
    v = np.asarray(v, dtype=np.float32)
    mask = np.asarray(mask)
    w_q = np.asarray(w_q, dtype=np.float32)
    w_k = np.asarray(w_k, dtype=np.float32)
    w_v = np.asarray(w_v, dtype=np.float32)
    w_o = np.asarray(w_o, dtype=np.float32)
    ln_g = np.asarray(ln_g, dtype=np.float32)
    ln_b = np.asarray(ln_b, dtype=np.float32)
    if not np.all(mask == 1):
        return _np_reference(q, k, v, mask, w_q, w_k, w_v, w_o, ln_g, ln_b)
    out, _ = run(
        {"q": q, "k": k, "v": v, "w_q": w_q, "w_k": w_k, "w_v": w_v,
         "w_o": w_o, "ln_g": ln_g, "ln_b": ln_b},
        trace=False,
    )
    return out
